# revision 57
# baseline (speedup 1.0000x reference)
"""Trainium2 Bass kernel for BaseLayerWithLoRA: out = x @ W.T + b + (x @ A.T) @ B.T.

Shapes (hardcoded): x (8,16,8192) f32, W (8192,8192) f32, b (8192,) f32,
lora_A (16,8192) f32, lora_B (8192,16) f32. Output (8,16,8192) f32.

Strategy: tensor-parallel over out_features (Dout=8192) across 8 cores,
1024 outputs per core; x replicated. The LoRA update is merged on host
(W' = W + B @ A — exact) so the device runs a single dense GEMM + bias.
Both operands are quantized to float8_e3m4 (4 mantissa bits) with fixed
power-of-2 scales (W'*128, x*2); the product descale 1/256 is applied on
host after the gather and the bias is pre-scaled to match. This
halves W DMA traffic vs fp16 AND halves the x load, giving the DMA
stream enough slack to stay ahead of the PE in every device clock mode.
Measured rel err 1.5325e-2 (deterministic) vs the 2e-2 gate.

Per core the stream is k-interleaved across two PSUM banks (out columns
0:512 and 512:1024) so one pass over the 64 k-tiles finishes both banks;
bank0 leads by 3 group-slots so its output DMAs overlap bank1's tail.
All inputs ride one sync-ring DMA conveyor in exactly the order the PE
consumes them: the HWDGE completion pipeline retires one 512KB/4KB-elem
chunk every ~1.27us (~410 GB/s) but only ~140 GB/s for small-elem
transfers, so xt rides in three large pieces slotted between W chunks
(the rings share 8 in-flight completion sems, so a second ring racing
ahead only delays the critical W chunks). The bias row is replicated
across partitions once by gpsimd (partition_broadcast, off the critical
path) and added for free in the PSUM->SBUF copies (tensor_tensor add),
so the PE stream carries zero bias matmuls; the k==0 matmul seeds each
bank and the single descale happens on host after the gather. The
chip grants full clock (~2.4 GHz) only after ~4us of sustained activity
and for at most ~34us before dropping to half util, so warmup matmuls
hold the PE busy from the entry barrier until the first W chunk's
completion (~11us) and everything else minimizes time-to-last-byte:
f32 outputs (bigger DMA elems), the final two 256-col pieces on two
independent engine+ring chains, and a TileContext exit reduced to a
bare sync drain (NRT's own postamble already rearms DMA and resets
semaphores). Measured: 74.4us (orig) -> 48.3us (prev session) ->
46.0us best observed; ~49-56us when the device's DVFS credit pool is
drained by back-to-back runs. rel err 1.5324e-2 (deterministic).
"""

import sys

for p in ("/opt/trn_rl_repo",):
    if p not in sys.path:
        sys.path.insert(0, p)

import ml_dtypes
import numpy as np

import concourse.bacc as bacc
import concourse.bass as bass
import concourse.mybir as mybir
import concourse.tile as tile
from concourse.bass_utils import run_bass_kernel_spmd


def _ensure_axon_hooks_stub():
    """run_bass_kernel_spmd imports antenv.axon_hooks when BASS_TRACE is set;
    this container's antenv stub lacks it. Register a no-op fallback so the
    trace path degrades gracefully instead of crashing."""
    try:
        import antenv.axon_hooks  # noqa: F401
    except ImportError:
        import types

        import antenv

        mod = types.ModuleType("antenv.axon_hooks")
        _hook = [None]
        mod.get_axon_ntff_profile_hook = lambda: _hook[0]
        mod.set_axon_ntff_profile_hook = lambda h: _hook.__setitem__(0, h)
        sys.modules["antenv.axon_hooks"] = mod
        antenv.axon_hooks = mod


_ensure_axon_hooks_stub()


def _trim_exit_barrier():
    """Reduce TileContext's exit sequence to a bare sync-engine drain.
    The NRT postamble already runs its own sync_barrier + sema_reset (51
    sems/engine) + dma_rearm after every execution, so the in-kernel
    all-engine barrier and gpsimd semaphore range-clear are redundant; the
    drain (with waits for every sem's final value) is kept so all DMA
    transfers — including the output writes — are complete before the
    engines' streams end. Idempotent, process-local."""
    from concourse.vector_clock import ScopedClock

    if getattr(tile.TileContext, "_exit_barrier_trimmed", False):
        return

    def _drain_and_barrier(self, tick_clock, wait_clock):
        drain_inst = self.nc.sync.drain()
        wait_clock.add_sem_waits(
            drain_inst.ins, ScopedClock({None: tick_clock.global_clock})
        )
        popped = self.nc._tile_sem_poison_stack.pop()
        assert popped is self._sem_poison

    tile.TileContext._drain_and_barrier = _drain_and_barrier
    tile.TileContext._exit_barrier_trimmed = True


_trim_exit_barrier()

# Problem constants
T = 128          # tokens = 8*16
DIN = 8192
DOUT = 8192
R = 16           # lora rank
NCORES = 8
DC = DOUT // NCORES      # 1024 out-features per core
KT = DIN // 128          # 64 k-tiles
KG = 8                   # k-tiles per W chunk (0.52 MB — keeps DMA issue-rate off the critical path)
G = KT // KG             # 8 groups per bank
XP = (16, 16, 32)        # xt piece sizes in k-tiles. The DMA completion
                         # conveyor retires ~410 GB/s only for >=2KB
                         # per-partition elements; small-elem transfers run
                         # at ~140 GB/s and stall the whole pipeline, so xt
                         # moves in three large pieces (2-4KB elems) placed
                         # at consumption-matched slots in the W stream
NWARM512A = 8            # fp8-rhs warmups bridging entry (~7.6us) to the
NWARM128A = 3            # first W chunk's completion (~12us); they keep the
                         # PE continuously busy so the DVFS grant (~12us,
                         # needs ~4us of sustained activity) lands as early
                         # as possible — the grant gates the whole stream,
                         # and full clock lasts exactly 34.1us before the
                         # chip drops to half-util, so the entire kernel
                         # (postamble included) must fit inside the window
LEAD = 3                 # bank0 group-slots of lead over bank1
SX = 2.0                 # x quantization scale (e3m4)
SW = 128.0               # W' quantization scale (e3m4)
OSCALE = 1.0 / (SX * SW)  # PSUM -> output descale (exact power of 2)
F16 = mybir.dt.float16
F8 = mybir.dt.float8e3
F32 = mybir.dt.float32

_CACHE = {}
LAST_RESULT = None


def build_bass():
    nc = bacc.Bacc("TRN2", target_bir_lowering=False)
    # xt[p, k, t] = (x * SX)[t, 128k+p] e3m4 — stationary operand tiles
    xt_d = nc.dram_tensor("xt", [128, KT, T], F8, kind="ExternalInput")
    # wt[bank, g, p, s*512+n] = (W' * 2^s)[DC*i + 512*bank + n, 128*(KG*g+s)+p]
    wt_d = nc.dram_tensor("wt", [2, G, 128, KG * 512], F8, kind="ExternalInput")
    bo_d = nc.dram_tensor("bo", [1, DC], F16, kind="ExternalInput")
    # f32 output: larger per-partition DMA elements than f16 for the
    # latency-bound tail transfers; the single descale happens on host
    out_d = nc.dram_tensor("out", [T, DC], F32, kind="ExternalOutput")

    with tile.TileContext(nc) as tc:
        with (
            tc.tile_pool(name="res", bufs=1) as res,
            tc.tile_pool(name="ps", bufs=1, space="PSUM") as ps,
        ):
            xt_s = res.tile([128, KT, T], F8)
            wt_s = res.tile([128, 2, G, KG * 512], F8)
            bo_s = res.tile([1, DC], F16)
            bob = res.tile([T, DC], F16)
            warm = res.tile([128, 512], F8)
            warmL = res.tile([128, T], F16)
            outs = res.tile([T, DC], F32)
            psum = [
                ps.tile([T, 512], F32, tag="p0", name="psum0"),
                ps.tile([T, 512], F32, tag="p1", name="psum1"),
            ]
            pwarm = ps.tile([T, 512], F32, tag="pw", name="psumw")

            # memsets on gpsimd: they land right after the entry barrier so
            # the PE warmups (and the DVFS activity clock) start sooner
            nc.gpsimd.memset(warm[:, :], 0.0)
            nc.gpsimd.memset(warmL[:, :], 0.0)

            # --- DMA program -------------------------------------------------
            xoff = [0]
            for n in XP:
                xoff.append(xoff[-1] + n)

            def xt_piece(i, eng):
                eng.dma_start(
                    out=xt_s[:, xoff[i] : xoff[i + 1], :],
                    in_=xt_d[:, xoff[i] : xoff[i + 1], :],
                )

            def w_chunk(bank, j):
                nc.sync.dma_start(out=wt_s[:, bank, j, :], in_=wt_d[bank, j])



            # One sync-ring conveyor in consumption order. The completion
            # pipeline retires one 512KB/4KB-elem chunk every ~1.27us; xt
            # pieces are large (high-efficiency) and slotted so each
            # completes well before the groups that read it. Only the tiny
            # bias rides the scalar ring.
            nc.scalar.dma_start(out=bo_s[:], in_=bo_d[:, :])
            # bias row replicated across partitions early (off the critical
            # path) so the PSUM->SBUF copies can add it for free — no bias
            # matmuls on the PE at all
            nc.gpsimd.partition_broadcast(bob[:, :], bo_s[:, :])
            xt_piece(0, nc.sync)       # k 0-15, feeds groups 0-1
            w_chunk(0, 0)
            w_chunk(0, 1)
            xt_piece(1, nc.sync)       # k 16-31, feeds groups 2-3
            w_chunk(0, 2)
            w_chunk(0, 3)
            w_chunk(1, 0)
            w_chunk(0, 4)
            w_chunk(1, 1)
            xt_piece(2, nc.sync)       # k 32-63, feeds groups 4-7
            w_chunk(0, 5)
            w_chunk(1, 2)
            for j in range(6, G):
                w_chunk(0, j)
                w_chunk(1, j - LEAD)
            for j in range(G - LEAD, G):
                w_chunk(1, j)

            # --- PE program --------------------------------------------------
            # Warmups (no DMA dependency) keep the PE busy while the front of
            # the stream lands, completing the p-state ramp; the N=128 warm
            # tail limits overshoot past the first W chunk's arrival.
            def warm512(n, start=False):
                for w in range(n):
                    nc.tensor.matmul(
                        pwarm[:], warmL[:, :], warm[:, :],
                        start=(start and w == 0), stop=False,
                        skip_group_check=True,
                    )

            def warm128(n, stop=False):
                for w in range(n):
                    nc.tensor.matmul(
                        pwarm[:, 0:T], warmL[:, :], warm[:, 0:T],
                        start=False, stop=(stop and w == n - 1),
                        skip_group_check=True,
                    )

            warm512(NWARM512A, start=True)
            warm128(NWARM128A, stop=True)

            def mm_group(bank, j):
                for s in range(KG):
                    k = KG * j + s
                    nc.tensor.matmul(
                        psum[bank][:], xt_s[:, k, :],
                        wt_s[:, bank, j, s * 512 : (s + 1) * 512],
                        start=(k == 0), stop=(k == KT - 1),
                        skip_group_check=True,
                    )

            # slot order: bank0 leads by LEAD groups, then alternate, then
            # bank1 drains — bank0's copies/output DMAs overlap bank1's tail.
            # The k==0 matmul seeds each PSUM bank (start=True).
            for j in range(LEAD):
                mm_group(0, j)
            for j in range(LEAD, G):
                mm_group(0, j)
                mm_group(1, j - LEAD)
            for piece in range(2):
                sl = slice(piece * 256, (piece + 1) * 256)
                nc.vector.tensor_tensor(
                    outs[:, sl], psum[0][:, sl], bob[:, sl], mybir.AluOpType.add
                )
                nc.scalar.dma_start(out=out_d[:, sl], in_=outs[:, sl])
            for j in range(G - LEAD, G):
                mm_group(1, j)
            # final bank: two 256-col pieces, copies back-to-back on vector
            # (bias-add fused), DMAs on different rings so the issue and the
            # ~1.4us transfer+sem latency overlap across the chains
            nc.vector.tensor_tensor(
                outs[:, 512:768], psum[1][:, 0:256], bob[:, 512:768],
                mybir.AluOpType.add,
            )
            nc.sync.dma_start(out=out_d[:, 512:768], in_=outs[:, 512:768])
            nc.vector.tensor_tensor(
                outs[:, 768:1024], psum[1][:, 256:512], bob[:, 768:1024],
                mybir.AluOpType.add,
            )
            nc.scalar.dma_start(out=out_d[:, 768:1024], in_=outs[:, 768:1024])

    nc.compile()
    return nc


def _prep_inputs(x, W, b, lora_A, lora_B):
    xf = np.asarray(x, dtype=np.float32).reshape(T, DIN)
    Wp = np.asarray(W, np.float32) + np.asarray(lora_B, np.float32) @ np.asarray(
        lora_A, np.float32
    )
    # fixed power-of-2 scales keep both operands inside e3m4's finite range
    # (+/-15.5); the product descale 1/(SX*SW) is applied on host
    W8 = np.clip(Wp * SW, -15.5, 15.5).astype(ml_dtypes.float8_e3m4)
    x8 = np.clip(xf * SX, -15.5, 15.5).astype(ml_dtypes.float8_e3m4)
    xt = np.ascontiguousarray(
        x8.view(np.uint8).reshape(T, KT, 128).transpose(2, 1, 0)
    ).view(ml_dtypes.float8_e3m4)
    # bias pre-scaled so the shared descale recovers it exactly
    b16 = (np.asarray(b, np.float32).astype(np.float16)) * np.float16(SX * SW)
    W8u = W8.view(np.uint8)
    in_maps = []
    for i in range(NCORES):
        sl = slice(i * DC, (i + 1) * DC)
        # wt[bank, g, p, s*512+n] = W8[DC*i + 512*bank + n, 128*(KG*g+s)+p]
        wt = np.ascontiguousarray(
            W8u[sl, :].T.reshape(G, KG, 128, 2, 512)
            .transpose(3, 0, 2, 1, 4)
            .reshape(2, G, 128, KG * 512)
        ).view(ml_dtypes.float8_e3m4)
        bo = np.ascontiguousarray(b16[sl].reshape(1, DC))
        in_maps.append({"xt": xt, "wt": wt, "bo": bo})
    return in_maps


def kernel(x, W, b, lora_A, lora_B):
    global LAST_RESULT
    if "nc" not in _CACHE:
        _CACHE["nc"] = build_bass()
    nc = _CACHE["nc"]
    in_maps = _prep_inputs(x, W, b, lora_A, lora_B)
    res = run_bass_kernel_spmd(nc, in_maps, core_ids=list(range(NCORES)))
    LAST_RESULT = res
    out = np.concatenate(
        [np.asarray(res.results[i]["out"]) for i in range(NCORES)], axis=1
    )
    # PSUM carries SX*SW*(x@W'.T) + SX*SW*b; one host-side descale recovers out
    out = out.astype(np.float32) * np.float32(OSCALE)
    return np.ascontiguousarray(out.reshape(8, 16, DOUT))



# revision 60
# speedup vs baseline: 1.0647x; 1.0647x over previous
"""Trainium2 Bass kernel for BaseLayerWithLoRA: out = x @ W.T + b + (x @ A.T) @ B.T.

Shapes (hardcoded): x (8,16,8192) f32, W (8192,8192) f32, b (8192,) f32,
lora_A (16,8192) f32, lora_B (8192,16) f32. Output (8,16,8192) f32.

Strategy: tensor-parallel over out_features (Dout=8192) across 8 cores, 1024
outputs per core; x replicated. LoRA is merged on host (W' = W + B @ A, exact)
so the device runs one dense GEMM + bias. Both operands are float8_e3m4 with
fixed power-of-2 scales (W'*128, x*2); the 1/256 descale rides the PSUM->SBUF
copies; bias is a K=1 ones-row matmul that seeds each accumulation group.
Measured rel err 1.5325e-2 vs the 2e-2 gate, deterministic.

Per core the stream is k-interleaved across two PSUM banks (columns 0:512 /
512:1024), bank0 leading by 3 group-slots so its copies and output DMAs
overlap bank1's tail. All inputs ride one sync-ring DMA stream in exactly the
order the PE consumes them (the HWDGE rings share ~8 in-flight completion
sems, and the queue FIFO follows issue order, so a second ring racing ahead
only delays the critical W chunks). 14 fp8 warmup matmuls bridge the DMA
front so the DVFS clock ramp completes before real work arrives and activity
stays sustained across the device's throttle duty-cycling. The TileContext
exit is reduced to a bare sync drain — NRT's own postamble already runs a
sync_barrier + 51-sem/engine reset + dma_rearm, so the in-kernel barrier and
gpsimd range-clear were ~1.7us of pure overhead (repeat executions validated
correct). A device-state-controlled A/B (alternating in one process) showed
this schedule beats a conveyor-optimized redesign (thin warmup, fused-bias
copies, f32 outs) by ~3us on a throttle-cycling device — the heavy warmup's
sustained activity is load-bearing; see kernel_redesign_backup.py and the
project memory notes for the full tried-and-failed list.

Measured: 74.4us (original) -> 48.2us (prev session) -> 46.0-46.6us with this
configuration across fresh and tired device states."""

import sys

for p in ("/opt/trn_rl_repo",):
    if p not in sys.path:
        sys.path.insert(0, p)

import ml_dtypes
import numpy as np

import concourse.bacc as bacc
import concourse.bass as bass
import concourse.mybir as mybir
import concourse.tile as tile
from concourse.bass_utils import run_bass_kernel_spmd


def _ensure_axon_hooks_stub():
    try:
        import antenv.axon_hooks  # noqa: F401
    except ImportError:
        import types

        import antenv

        mod = types.ModuleType("antenv.axon_hooks")
        _hook = [None]
        mod.get_axon_ntff_profile_hook = lambda: _hook[0]
        mod.set_axon_ntff_profile_hook = lambda h: _hook.__setitem__(0, h)
        sys.modules["antenv.axon_hooks"] = mod
        antenv.axon_hooks = mod


_ensure_axon_hooks_stub()


def _trim_exit_barrier():
    """Reduce TileContext's exit sequence to a bare sync-engine drain.
    The NRT postamble already runs its own sync_barrier + sema_reset (51
    sems/engine) + dma_rearm after every execution, so the in-kernel
    all-engine barrier and gpsimd semaphore range-clear are redundant; the
    drain (with waits for every sem's final value) is kept so all DMA
    transfers — including the output writes — are complete before the
    engines' streams end. Idempotent, process-local."""
    from concourse.vector_clock import ScopedClock

    if getattr(tile.TileContext, "_exit_barrier_trimmed", False):
        return

    def _drain_and_barrier(self, tick_clock, wait_clock):
        drain_inst = self.nc.sync.drain()
        wait_clock.add_sem_waits(
            drain_inst.ins, ScopedClock({None: tick_clock.global_clock})
        )
        popped = self.nc._tile_sem_poison_stack.pop()
        assert popped is self._sem_poison

    tile.TileContext._drain_and_barrier = _drain_and_barrier
    tile.TileContext._exit_barrier_trimmed = True


_trim_exit_barrier()

T = 128
DIN = 8192
DOUT = 8192
R = 16
NCORES = 8
DC = DOUT // NCORES
KT = DIN // 128
KG = 8
G = KT // KG
XP = (8, 8, 16, 16, 16)
NWARM512A = 6
NWARM128A = 2
NWARM512B = 4
NWARM128B = 2
LEAD = 3
SX = 2.0
SW = 128.0
OSCALE = 1.0 / (SX * SW)
F16 = mybir.dt.float16
F8 = mybir.dt.float8e3
F32 = mybir.dt.float32

_CACHE = {}
LAST_RESULT = None


def build_bass():
    nc = bacc.Bacc("TRN2", target_bir_lowering=False)
    xt_d = nc.dram_tensor("xt", [128, KT, T], F8, kind="ExternalInput")
    wt_d = nc.dram_tensor("wt", [2, G, 128, KG * 512], F8, kind="ExternalInput")
    bo_d = nc.dram_tensor("bo", [1, DC], F16, kind="ExternalInput")
    out_d = nc.dram_tensor("out", [T, DC], F16, kind="ExternalOutput")

    with tile.TileContext(nc) as tc:
        with (
            tc.tile_pool(name="res", bufs=1) as res,
            tc.tile_pool(name="ps", bufs=1, space="PSUM") as ps,
        ):
            xt_s = res.tile([128, KT, T], F8)
            wt_s = res.tile([128, 2, G, KG * 512], F8)
            bo_s = res.tile([1, DC], F16)
            ones = res.tile([1, T], F16)
            warm = res.tile([128, 512], F8)
            warmL = res.tile([128, T], F16)
            outs = res.tile([T, DC], F16)
            psum = [
                ps.tile([T, 512], F32, tag="p0", name="psum0"),
                ps.tile([T, 512], F32, tag="p1", name="psum1"),
            ]
            pwarm = ps.tile([T, 512], F32, tag="pw", name="psumw")

            nc.vector.memset(warm[:, :], 0.0)
            nc.vector.memset(warmL[:, :], 0.0)
            nc.vector.memset(ones[:, :], 1.0)

            xoff = [0]
            for n in XP:
                xoff.append(xoff[-1] + n)

            def xt_piece(i, eng):
                eng.dma_start(
                    out=xt_s[:, xoff[i] : xoff[i + 1], :],
                    in_=xt_d[:, xoff[i] : xoff[i + 1], :],
                )

            def w_chunk(bank, j):
                nc.sync.dma_start(out=wt_s[:, bank, j, :], in_=wt_d[bank, j])

            nc.scalar.dma_start(out=bo_s[:], in_=bo_d[:, :])
            xt_piece(0, nc.sync)
            w_chunk(0, 0)
            xt_piece(1, nc.sync)
            w_chunk(0, 1)
            xt_piece(2, nc.sync)
            w_chunk(0, 2)
            w_chunk(0, 3)
            w_chunk(1, 0)
            w_chunk(0, 4)
            xt_piece(3, nc.sync)
            w_chunk(1, 1)
            w_chunk(0, 5)
            xt_piece(4, nc.sync)
            w_chunk(1, 2)
            for j in range(6, G):
                w_chunk(0, j)
                w_chunk(1, j - LEAD)
            for j in range(G - LEAD, G):
                w_chunk(1, j)

            def warm512(n, start=False):
                for w in range(n):
                    nc.tensor.matmul(
                        pwarm[:], warmL[:, :], warm[:, :],
                        start=(start and w == 0), stop=False,
                        skip_group_check=True,
                    )

            def warm128(n, stop=False):
                for w in range(n):
                    nc.tensor.matmul(
                        pwarm[:, 0:T], warmL[:, :], warm[:, 0:T],
                        start=False, stop=(stop and w == n - 1),
                        skip_group_check=True,
                    )

            warm512(NWARM512A, start=True)
            warm128(NWARM128A)
            warm512(NWARM512B)

            def mm_bias(bank):
                nc.tensor.matmul(
                    psum[bank][:], ones[:, :],
                    bo_s[:, bank * 512 : (bank + 1) * 512],
                    start=True, stop=False, skip_group_check=True,
                )

            warm128(NWARM128B, stop=True)
            mm_bias(0)
            mm_bias(1)

            def mm_group(bank, j):
                for s in range(KG):
                    k = KG * j + s
                    nc.tensor.matmul(
                        psum[bank][:], xt_s[:, k, :],
                        wt_s[:, bank, j, s * 512 : (s + 1) * 512],
                        start=False, stop=(k == KT - 1),
                        skip_group_check=True,
                    )

            for j in range(LEAD):
                mm_group(0, j)
            for j in range(LEAD, G):
                mm_group(0, j)
                mm_group(1, j - LEAD)
            for piece in range(2):
                sl = slice(piece * 256, (piece + 1) * 256)
                nc.vector.tensor_scalar_mul(outs[:, sl], psum[0][:, sl], OSCALE)
                nc.scalar.dma_start(out=out_d[:, sl], in_=outs[:, sl])
            for j in range(G - LEAD, G):
                mm_group(1, j)
            nc.vector.tensor_scalar_mul(outs[:, 512:896], psum[1][:, 0:384], OSCALE)
            nc.sync.dma_start(out=out_d[:, 512:896], in_=outs[:, 512:896])
            nc.vector.tensor_scalar_mul(outs[:, 896:1024], psum[1][:, 384:512], OSCALE)
            nc.scalar.dma_start(out=out_d[:, 896:1024], in_=outs[:, 896:1024])

    nc.compile()
    return nc


def _prep_inputs(x, W, b, lora_A, lora_B):
    xf = np.asarray(x, dtype=np.float32).reshape(T, DIN)
    Wp = np.asarray(W, np.float32) + np.asarray(lora_B, np.float32) @ np.asarray(
        lora_A, np.float32
    )
    W8 = np.clip(Wp * SW, -15.5, 15.5).astype(ml_dtypes.float8_e3m4)
    x8 = np.clip(xf * SX, -15.5, 15.5).astype(ml_dtypes.float8_e3m4)
    xt = np.ascontiguousarray(
        x8.view(np.uint8).reshape(T, KT, 128).transpose(2, 1, 0)
    ).view(ml_dtypes.float8_e3m4)
    b16 = (np.asarray(b, np.float32).astype(np.float16)) * np.float16(SX * SW)
    W8u = W8.view(np.uint8)
    in_maps = []
    for i in range(NCORES):
        sl = slice(i * DC, (i + 1) * DC)
        wt = np.ascontiguousarray(
            W8u[sl, :].T.reshape(G, KG, 128, 2, 512)
            .transpose(3, 0, 2, 1, 4)
            .reshape(2, G, 128, KG * 512)
        ).view(ml_dtypes.float8_e3m4)
        bo = np.ascontiguousarray(b16[sl].reshape(1, DC))
        in_maps.append({"xt": xt, "wt": wt, "bo": bo})
    return in_maps


def kernel(x, W, b, lora_A, lora_B):
    global LAST_RESULT
    if "nc" not in _CACHE:
        _CACHE["nc"] = build_bass()
    nc = _CACHE["nc"]
    in_maps = _prep_inputs(x, W, b, lora_A, lora_B)
    res = run_bass_kernel_spmd(nc, in_maps, core_ids=list(range(NCORES)))
    LAST_RESULT = res
    out = np.concatenate(
        [np.asarray(res.results[i]["out"]) for i in range(NCORES)], axis=1
    )
    return np.ascontiguousarray(out.reshape(8, 16, DOUT)).astype(np.float32)


# revision 67
# speedup vs baseline: 1.0692x; 1.0043x over previous
"""Trainium2 Bass kernel for BaseLayerWithLoRA: out = x @ W.T + b + (x @ A.T) @ B.T.

Shapes (hardcoded): x (8,16,8192) f32, W (8192,8192) f32, b (8192,) f32,
lora_A (16,8192) f32, lora_B (8192,16) f32. Output (8,16,8192) f32.

Strategy: tensor-parallel over out_features (Dout=8192) across 8 cores, 1024
outputs per core; x replicated. LoRA is merged on host (W' = W + B @ A, exact)
so the device runs one dense GEMM + bias. Both operands are float8_e3m4 with
fixed power-of-2 scales (W'*128, x*2); the 1/256 descale rides the PSUM->SBUF
copies; bias is a K=1 ones-row matmul that seeds each accumulation group.
Measured rel err 1.5325e-2 vs the 2e-2 gate, deterministic.

Per core the stream is k-interleaved across two PSUM banks (columns 0:512 /
512:1024), bank0 leading by 3 group-slots so its copies and output DMAs
overlap bank1's tail. All inputs ride one sync-ring DMA stream in exactly the
order the PE consumes them (the HWDGE rings share ~8 in-flight completion
sems, and the queue FIFO follows issue order, so a second ring racing ahead
only delays the critical W chunks). 14 fp8 warmup matmuls bridge the DMA
front so the DVFS clock ramp completes before real work arrives and activity
stays sustained across the device's throttle duty-cycling. The TileContext
exit is reduced to a bare sync drain — NRT's own postamble already runs a
sync_barrier + 51-sem/engine reset + dma_rearm, so the in-kernel barrier and
gpsimd range-clear were ~1.7us of pure overhead (repeat executions validated
correct). A device-state-controlled A/B (alternating in one process) showed
this schedule beats a conveyor-optimized redesign (thin warmup, fused-bias
copies, f32 outs) by ~3us on a throttle-cycling device — the heavy warmup's
sustained activity is load-bearing; see kernel_redesign_backup.py and the
project memory notes for the full tried-and-failed list.

Measured: 74.4us (original) -> 48.2us (prev session) -> 46.0-46.6us with this
configuration across fresh and tired device states."""

import sys

for p in ("/opt/trn_rl_repo",):
    if p not in sys.path:
        sys.path.insert(0, p)

import ml_dtypes
import numpy as np

import concourse.bacc as bacc
import concourse.bass as bass
import concourse.mybir as mybir
import concourse.tile as tile
from concourse.bass_utils import run_bass_kernel_spmd


def _ensure_axon_hooks_stub():
    try:
        import antenv.axon_hooks  # noqa: F401
    except ImportError:
        import types

        import antenv

        mod = types.ModuleType("antenv.axon_hooks")
        _hook = [None]
        mod.get_axon_ntff_profile_hook = lambda: _hook[0]
        mod.set_axon_ntff_profile_hook = lambda h: _hook.__setitem__(0, h)
        sys.modules["antenv.axon_hooks"] = mod
        antenv.axon_hooks = mod


_ensure_axon_hooks_stub()


def _trim_exit_barrier():
    """Reduce TileContext's exit sequence to a bare sync-engine drain.
    The NRT postamble already runs its own sync_barrier + sema_reset (51
    sems/engine) + dma_rearm after every execution, so the in-kernel
    all-engine barrier and gpsimd semaphore range-clear are redundant; the
    drain (with waits for every sem's final value) is kept so all DMA
    transfers — including the output writes — are complete before the
    engines' streams end. Idempotent, process-local."""
    from concourse.vector_clock import ScopedClock

    if getattr(tile.TileContext, "_exit_barrier_trimmed", False):
        return

    def _drain_and_barrier(self, tick_clock, wait_clock):
        drain_inst = self.nc.sync.drain()
        wait_clock.add_sem_waits(
            drain_inst.ins, ScopedClock({None: tick_clock.global_clock})
        )
        popped = self.nc._tile_sem_poison_stack.pop()
        assert popped is self._sem_poison

    tile.TileContext._drain_and_barrier = _drain_and_barrier
    tile.TileContext._exit_barrier_trimmed = True


_trim_exit_barrier()

T = 128
DIN = 8192
DOUT = 8192
R = 16
NCORES = 8
DC = DOUT // NCORES
KT = DIN // 128
KG = 8
G = KT // KG
XP = (8, 8, 16, 16, 16)
NWARM512A = 6
NWARM128A = 2
NWARM512B = 4
NWARM128B = 2
LEAD = 3
SX = 2.0
SW = 128.0
OSCALE = 1.0 / (SX * SW)
F16 = mybir.dt.float16
F8 = mybir.dt.float8e3
F32 = mybir.dt.float32

_CACHE = {}
LAST_RESULT = None


def build_bass():
    nc = bacc.Bacc("TRN2", target_bir_lowering=False)
    xt_d = nc.dram_tensor("xt", [128, KT, T], F8, kind="ExternalInput")
    wt_d = nc.dram_tensor("wt", [2, G, 128, KG * 512], F8, kind="ExternalInput")
    bo_d = nc.dram_tensor("bo", [1, DC], F16, kind="ExternalInput")
    out_d = nc.dram_tensor("out", [T, DC], F16, kind="ExternalOutput")

    with tile.TileContext(nc) as tc:
        with (
            tc.tile_pool(name="res", bufs=1) as res,
            tc.tile_pool(name="ps", bufs=1, space="PSUM") as ps,
        ):
            xt_s = res.tile([128, KT, T], F8)
            wt_s = res.tile([128, 2, G, KG * 512], F8)
            bo_s = res.tile([1, DC], F16)
            ones = res.tile([1, T], F16)
            warm = res.tile([128, 512], F8)
            warmL = res.tile([128, T], F16)
            outs = res.tile([T, DC], F16)
            psum = [
                ps.tile([T, 512], F32, tag="p0", name="psum0"),
                ps.tile([T, 512], F32, tag="p1", name="psum1"),
            ]
            pwarm = ps.tile([T, 512], F32, tag="pw", name="psumw")

            nc.vector.memset(warm[:, :], 0.0)
            nc.vector.memset(warmL[:, :], 0.0)
            nc.vector.memset(ones[:, :], 1.0)

            xoff = [0]
            for n in XP:
                xoff.append(xoff[-1] + n)

            def xt_piece(i, eng):
                eng.dma_start(
                    out=xt_s[:, xoff[i] : xoff[i + 1], :],
                    in_=xt_d[:, xoff[i] : xoff[i + 1], :],
                )

            def w_chunk(bank, j):
                nc.sync.dma_start(out=wt_s[:, bank, j, :], in_=wt_d[bank, j])

            nc.scalar.dma_start(out=bo_s[:], in_=bo_d[:, :])
            xt_piece(0, nc.sync)
            w_chunk(0, 0)
            xt_piece(1, nc.sync)
            w_chunk(0, 1)
            xt_piece(2, nc.sync)
            w_chunk(0, 2)
            w_chunk(0, 3)
            w_chunk(1, 0)
            w_chunk(0, 4)
            xt_piece(3, nc.sync)
            w_chunk(1, 1)
            w_chunk(0, 5)
            xt_piece(4, nc.sync)
            w_chunk(1, 2)
            for j in range(6, G):
                w_chunk(0, j)
                w_chunk(1, j - LEAD)
            for j in range(G - LEAD, G):
                w_chunk(1, j)

            def warm512(n, start=False):
                for w in range(n):
                    nc.tensor.matmul(
                        pwarm[:], warmL[:, :], warm[:, :],
                        start=(start and w == 0), stop=False,
                        skip_group_check=True,
                    )

            def warm128(n, stop=False):
                for w in range(n):
                    nc.tensor.matmul(
                        pwarm[:, 0:T], warmL[:, :], warm[:, 0:T],
                        start=False, stop=(stop and w == n - 1),
                        skip_group_check=True,
                    )

            warm512(NWARM512A, start=True)
            warm128(NWARM128A)
            warm512(NWARM512B)

            def mm_bias(bank):
                nc.tensor.matmul(
                    psum[bank][:], ones[:, :],
                    bo_s[:, bank * 512 : (bank + 1) * 512],
                    start=True, stop=False, skip_group_check=True,
                )

            warm128(NWARM128B, stop=True)
            mm_bias(0)
            mm_bias(1)

            def mm_group(bank, j):
                for s in range(KG):
                    k = KG * j + s
                    nc.tensor.matmul(
                        psum[bank][:], xt_s[:, k, :],
                        wt_s[:, bank, j, s * 512 : (s + 1) * 512],
                        start=False, stop=(k == KT - 1),
                        skip_group_check=True,
                    )

            for j in range(LEAD):
                mm_group(0, j)
            for j in range(LEAD, G):
                mm_group(0, j)
                mm_group(1, j - LEAD)
            for piece in range(2):
                sl = slice(piece * 256, (piece + 1) * 256)
                nc.vector.tensor_scalar_mul(outs[:, sl], psum[0][:, sl], OSCALE)
                nc.scalar.dma_start(out=out_d[:, sl], in_=outs[:, sl])
            for j in range(G - LEAD, G):
                mm_group(1, j)
            nc.vector.tensor_scalar_mul(outs[:, 512:896], psum[1][:, 0:384], OSCALE)
            nc.sync.dma_start(out=out_d[:, 512:896], in_=outs[:, 512:896])
            nc.vector.tensor_scalar_mul(outs[:, 896:1024], psum[1][:, 384:512], OSCALE)
            nc.scalar.dma_start(out=out_d[:, 896:1024], in_=outs[:, 896:1024])

    nc.compile()
    return nc


def _prep_inputs(x, W, b, lora_A, lora_B):
    xf = np.asarray(x, dtype=np.float32).reshape(T, DIN)
    Wp = np.asarray(W, np.float32) + np.asarray(lora_B, np.float32) @ np.asarray(
        lora_A, np.float32
    )
    W8 = np.clip(Wp * SW, -15.5, 15.5).astype(ml_dtypes.float8_e3m4)
    x8 = np.clip(xf * SX, -15.5, 15.5).astype(ml_dtypes.float8_e3m4)
    xt = np.ascontiguousarray(
        x8.view(np.uint8).reshape(T, KT, 128).transpose(2, 1, 0)
    ).view(ml_dtypes.float8_e3m4)
    b16 = (np.asarray(b, np.float32).astype(np.float16)) * np.float16(SX * SW)
    W8u = W8.view(np.uint8)
    in_maps = []
    for i in range(NCORES):
        sl = slice(i * DC, (i + 1) * DC)
        wt = np.ascontiguousarray(
            W8u[sl, :].T.reshape(G, KG, 128, 2, 512)
            .transpose(3, 0, 2, 1, 4)
            .reshape(2, G, 128, KG * 512)
        ).view(ml_dtypes.float8_e3m4)
        bo = np.ascontiguousarray(b16[sl].reshape(1, DC))
        in_maps.append({"xt": xt, "wt": wt, "bo": bo})
    return in_maps


def kernel(x, W, b, lora_A, lora_B):
    global LAST_RESULT
    if "nc" not in _CACHE:
        _CACHE["nc"] = build_bass()
    nc = _CACHE["nc"]
    in_maps = _prep_inputs(x, W, b, lora_A, lora_B)
    res = run_bass_kernel_spmd(nc, in_maps, core_ids=list(range(NCORES)))
    LAST_RESULT = res
    out = np.concatenate(
        [np.asarray(res.results[i]["out"]) for i in range(NCORES)], axis=1
    )
    return np.ascontiguousarray(out.reshape(8, 16, DOUT)).astype(np.float32)
